# revision 30
# baseline (speedup 1.0000x reference)
"""Bass/Trainium2 kernel for nn_BiGRIL (gnn_message_passing).

Algebraic structure (h == 0, C == 1):
  u    = (x - bfs)*m                    (host-computed input prep)
  z    = W0*x1 + W1*m + b_in            (rank-2 in channels)
  zg   = A^T z  ->  W0*ug + W1*mg + b_in'*cg   with ug = A^T u, mg = A^T m
  v    = PA6^T [u,m,ug,mg,cg,1]         (K=6 matmul; bias via ones-row)
  o    = PReLU(v)                       (ACT Prelu, psum->fp16)
  w    = PB^T o                         (pass B; PB = wro1 (x) W_o1 rank-1)
  rr   = relu(w + bias_f)               (ACT Relu-bias / DVE TS, psum->fp16)
  out  = W_o2 . rr + b_o2               (pass C; +b_o2 via final copy bias)

Superstep schedule: 2 chunk-pairs per superstep; pass A fills a fused
[128,1024] 2-bank psum tile consumed by ONE wide Prelu; pass B fills a
single rotating [128,1024] psum tile consumed by ONE wide relu; pass C
packs its 4 useful output rows per superstep into ONE accumulated psum
bank (rows 4s..4s+3 via 16 stationary column-variants + tile_position),
so the whole output drains with a single copy + a single 128KB DMA.

Inputs are host-preshuffled: u/m/cg ma-rows arrive position-major straight
from HBM; only device-computed ug/mg need on-chip gathers (gpsimd swdge).
adj is grouped mt-major so G-tiles start as stripes land.  Output is
written chunk-major and unscrambled on host.

Sharding: data-parallel over batch (B=8 -> 8 cores), no collectives.
"""

import numpy as np
import sys

sys.path.insert(0, "/opt/trn_rl_repo")

B, C, N, T = 8, 1, 1024, 64
H = 64
NT = N * T          # 65536 per-core output elements
CHUNK = 512
NPAIR = 64          # pairs (g, g+64); chunk c=g is lane0, c=g+64 lane1
NSUP = 32           # supersteps of 2 pairs
HALF = NT // 2      # 32768: lane1 position offset
BLK = 4096          # ma tile columns (64 nodes x 64 steps)
NBLK = 8            # blocks per lane

_CACHE = {}


def _fold_weights(W_fs, b_fs, W_in, b_in, W_gc, b_gc, W_lo, b_lo, prelu_a,
                  W_ro, b_ro, W_o1, b_o1, W_o2, b_o2, adj):
    """Host-side weight folding in float64 for accuracy."""
    f8 = np.float64
    W_in, b_in = W_in.astype(f8), b_in.astype(f8)
    W_gc, b_gc = W_gc.astype(f8), b_gc.astype(f8)
    W_lo, b_lo = W_lo.astype(f8), b_lo.astype(f8)
    W_ro, b_ro = W_ro.astype(f8), b_ro.astype(f8)
    W_o1, b_o1 = W_o1.astype(f8), b_o1.astype(f8)
    W_o2, b_o2 = W_o2.astype(f8), b_o2.astype(f8)

    W0 = W_in[:, 0]           # x1 channel  [64]
    W1 = W_in[:, 1]           # mask channel [64]
    Wlo1 = W_lo[:, :H]
    M1 = Wlo1 @ W_gc[:, :H]
    M2 = Wlo1 @ W_gc[:, H:]
    b_fold = Wlo1 @ b_gc + b_lo

    b_in_p = b_in + W0 * float(b_fs[0])
    PA = np.stack([
        M1 @ W0,
        M1 @ W1,
        M2 @ W0,
        M2 @ W1,
        M2 @ b_in_p,
    ])                                     # [5, 64]  lhsT for pass A
    bias_v = M1 @ b_in_p + b_fold          # [64] -> ones-row of pass A

    w_ro1 = W_ro[0, :H]                    # [64]
    PB = np.outer(w_ro1, W_o1[:, 0])       # [64(h), 64(f)] lhsT for pass B
    bias_f = W_o1[:, 0] * b_ro[0] + b_o1   # [64]

    cg = adj.astype(f8).sum(axis=0)        # [N] column sums of adj
    cgrep = np.repeat(cg, T)               # [(n,t)] layout n*T + t

    a = float(prelu_a)
    assert 0.0 < a < 1.0

    # pass A stationary [11, 128]: row 0 = shared ones stream carrying
    # bias_v for BOTH lanes; rows 1-5 = lane0 streams (u,m,ug,mg,cg) ->
    # out cols 0:64; rows 6-10 = lane1 streams -> out cols 64:128.
    # K=11 means the moving tile has NO dead rows -> no zero-memsets.
    paA = np.zeros((128, 128))
    paA[0, 0:64] = bias_v
    paA[0, 64:128] = bias_v
    paA[1:6, 0:64] = PA
    paA[6:11, 64:128] = PA
    # pass B stationary: blockdiag(PB, PB) over packed prelu [128, 512]
    pbB = np.zeros((128, 128))
    pbB[0:64, 0:64] = PB
    pbB[64:128, 64:128] = PB
    # pass C stationaries: 16 variants [128, 32]; variant k = 2j+q places
    # lane0 at col 4j+2q, lane1 at col 4j+2q+1 -> accumulated po-bank row
    # 32t + 4j + 2q + lane for superstep s = 8t+j, pair q
    pc = np.zeros((128, 16 * 32))
    for k in range(16):
        j, q = k // 2, k % 2
        pc[0:64, 32 * k + 4 * j + 2 * q] = W_o2[0]
        pc[64:128, 32 * k + 4 * j + 2 * q + 1] = W_o2[0]

    h16 = np.float16
    fp = np.float32
    ch = np.zeros((128, 768))
    ch[:, 0:128] = paA
    ch[:, 128:256] = pbB
    ch[:, 256:768] = pc
    cf = np.zeros((128, 4))
    cf[0:64, 0] = bias_f           # rr relu bias
    cf[64:128, 0] = bias_f
    cf[:, 1] = float(b_o2[0])      # final copy bias (all rows are outputs)
    cf[:, 2] = a                   # prelu slope (unused; ACT alpha)
    # col 3 stays zero: zero-scalar operand for DVE tensor_scalar ops

    return dict(
        consts_h=ch.astype(h16),
        consts_f=cf.astype(fp),
        cgrep=cgrep.astype(h16),
        prelu_a=a,
    )


def _build_program(prelu_a):
    import concourse.bass as bass
    import concourse.bacc as bacc
    import concourse.mybir as mybir
    import concourse.tile as tile

    dt = mybir.dt
    f32 = dt.float32
    h16 = dt.float16
    AF = mybir.ActivationFunctionType
    ALU = mybir.AluOpType

    nc = bacc.Bacc("TRN2", target_bir_lowering=False, debug=False,
                   num_devices=B)

    um_d = nc.dram_tensor("um", [128, 1024], h16, kind="ExternalInput")
    ones_d = nc.dram_tensor("ones", [BLK], h16, kind="ExternalInput")
    up_d = nc.dram_tensor("up", [NT], h16, kind="ExternalInput")
    mp_d = nc.dram_tensor("mp", [NT], h16, kind="ExternalInput")
    cg_d = nc.dram_tensor("cgrep", [NT], h16, kind="ExternalInput")
    adjs = nc.dram_tensor("adjs", [128, 8192], h16, kind="ExternalInput")
    ch_d = nc.dram_tensor("consts_h", [128, 768], h16, kind="ExternalInput")
    cf_d = nc.dram_tensor("consts_f", [128, 4], f32, kind="ExternalInput")
    # row-major [128, 512] final tile; row 4s+2q+lane = superstep s pair q
    out_d = nc.dram_tensor("out", [NT], h16, kind="ExternalOutput")

    MOVA_BUFS = 4
    from contextlib import ExitStack
    with tile.TileContext(nc) as tc, ExitStack() as ctx:
        const = ctx.enter_context(tc.tile_pool(name="const", bufs=1))
        adjp = ctx.enter_context(tc.tile_pool(name="adjp", bufs=1))
        ump = ctx.enter_context(tc.tile_pool(name="ump", bufs=1))
        gxp = ctx.enter_context(tc.tile_pool(name="gxp", bufs=1))
        movap = ctx.enter_context(tc.tile_pool(name="movap", bufs=1))
        op_p = ctx.enter_context(tc.tile_pool(name="op", bufs=3))
        rrp = ctx.enter_context(tc.tile_pool(name="rrp", bufs=2))
        posbp = ctx.enter_context(tc.tile_pool(name="posbp", bufs=1))
        # PSUM budget (8 banks): vps 2x2 + wps 1x2 + pop 1 (+1 spare)
        vps = ctx.enter_context(tc.tile_pool(name="vps", bufs=2, space="PSUM"))
        wps = ctx.enter_context(tc.tile_pool(name="wps", bufs=1, space="PSUM"))
        pop = ctx.enter_context(tc.tile_pool(name="pop", bufs=1, space="PSUM"))

        # ---- consolidated constant loads (2 DMAs, first on sync) ------
        ch_t = const.tile([128, 768], h16)
        cf_t = const.tile([128, 4], f32)
        nc.sync.dma_start(out=ch_t[:], in_=ch_d[:])
        nc.sync.dma_start(out=cf_t[:], in_=cf_d[:])
        pa_t = ch_t[0:11, 0:128]
        pb_t = ch_t[:, 128:256]
        pc_t = [ch_t[:, 256 + 32 * k:256 + 32 * (k + 1)] for k in range(16)]
        bf_t = cf_t[:, 0:1]
        bo2_t = cf_t[:, 1:2]
        zero_t = cf_t[:, 3:4]

        # ---- G moving operand: [u|m] node-major, one DMA ---------------
        um_t = ump.tile([128, 1024], h16, tag="um", name="um")
        nc.sync.dma_start(out=um_t[:], in_=um_d[:])
        gmh = um_t[:, :].rearrange("p (h q) -> p h q", h=2)

        # ---- adj: 8 mt-stripes; 0,4 now, rest staggered in st_a --------
        adjt = adjp.tile([128, 8192], h16, tag="adjt", name="adjt")

        def load_stripe(mt, eng):
            c0 = mt * 1024
            eng.dma_start(out=adjt[:, c0:c0 + 1024],
                          in_=adjs[:, c0:c0 + 1024])

        load_stripe(0, nc.sync)
        load_stripe(4, nc.scalar)

        # ---- HAM warmup into the pop bank (garbage harmless: each 32-row
        # group is zeroed by its first C matmul's start=True) -------------
        po_ps = pop.tile([128, 512], f32, tag="po", name="po")
        for wi in range(28):
            nc.tensor.matmul(po_ps[:, 0:128], pb_t, pb_t,
                             start=True, stop=True, skip_group_check=True)

        # ---- ma tiles: 4 rotating [11, 4096] fp16 ----------------------
        # row 0 = shared ones (DMA-loaded once); rows 1-5 lane0 streams
        # (u, m, ug, mg, cg); rows 6-10 lane1.  No dead rows, no memsets.
        ma4 = [movap.tile([11, BLK], h16, tag=f"mova{i}", name=f"mova{i}")
               for i in range(MOVA_BUFS)]
        for i in range(MOVA_BUFS):
            nc.sync.dma_start(out=ma4[i][0:1, :], in_=ones_d[:])

        gx = [None] * 8
        ma_t = [None] * NBLK

        def emit_g(mt):
            # G psum borrows a vps tile (cols 0:128 used)
            psg = vps.tile([128, 1024], f32, tag="v", name=f"psg{mt}")
            for nt in range(8):
                c0 = mt * 1024 + nt * 128
                nc.tensor.matmul(
                    psg[:, 0:128],
                    adjt[:, c0:c0 + 128],
                    gmh[:, :, nt * 64:(nt + 1) * 64],
                    start=(nt == 0), stop=(nt == 7))
            g = gxp.tile([128, 128], h16, tag=f"gxm{mt}", name=f"gxm{mt}")
            nc.scalar.activation(g[:], psg[:, 0:128], AF.Copy,
                                 bias=0.0, scale=1.0)
            gx[mt] = g

        def emit_ma(blk):
            mt0, mt1 = blk // 2, 4 + blk // 2
            p0 = (blk % 2) * 64
            ma = ma4[blk % MOVA_BUFS]
            o0, o1 = blk * BLK, HALF + blk * BLK
            # position-major HBM rows (u, m, cg per lane)
            nc.sync.dma_start(out=ma[1:2, :], in_=up_d[o0:o0 + BLK])
            nc.scalar.dma_start(out=ma[2:3, :], in_=mp_d[o0:o0 + BLK])
            nc.sync.dma_start(out=ma[5:6, :], in_=cg_d[o0:o0 + BLK])
            nc.sync.dma_start(out=ma[6:7, :], in_=up_d[o1:o1 + BLK])
            nc.scalar.dma_start(out=ma[7:8, :], in_=mp_d[o1:o1 + BLK])
            nc.sync.dma_start(out=ma[10:11, :], in_=cg_d[o1:o1 + BLK])
            # device-computed ug/mg gathers (split scalar hw-DGE / gpsimd)
            nc.scalar.dma_start(out=ma[3:4, :], in_=gx[mt0][p0:p0 + 64, 0:64])
            nc.scalar.dma_start(out=ma[4:5, :],
                                in_=gx[mt0][p0:p0 + 64, 64:128])
            nc.gpsimd.dma_start(out=ma[8:9, :],
                                in_=gx[mt1][p0:p0 + 64, 0:64])
            nc.gpsimd.dma_start(out=ma[9:10, :],
                                in_=gx[mt1][p0:p0 + 64, 64:128])
            ma_t[blk] = ma

        o_t = {}
        rr_t = {}

        def st_a(s):
            blk = s // 4
            if s == 0:
                emit_g(0)
                emit_g(4)
                emit_ma(0)
                emit_ma(1)
                load_stripe(1, nc.scalar)
                load_stripe(5, nc.sync)
            elif s == 1:
                load_stripe(2, nc.scalar)
                load_stripe(6, nc.sync)
            elif s == 2:
                load_stripe(3, nc.scalar)
                load_stripe(7, nc.sync)
            if s % 4 == 2 and blk % 2 == 0 and blk // 2 + 1 < 4:
                emit_g(blk // 2 + 1)
                emit_g(4 + blk // 2 + 1)
            if s % 4 == 3 and blk + 2 < NBLK:
                emit_ma(blk + 2)
            c0 = (s % 4) * 1024
            vt = vps.tile([128, 1024], f32, tag="v", name=f"v{s}")
            nc.tensor.matmul(vt[:, 0:512], pa_t,
                             ma_t[blk][:, c0:c0 + 512],
                             start=True, stop=True)
            nc.tensor.matmul(vt[:, 512:1024], pa_t,
                             ma_t[blk][:, c0 + 512:c0 + 1024],
                             start=True, stop=True)
            o = op_p.tile([128, 1024], h16, tag="o", name=f"o{s}")
            nc.scalar.activation(o[:], vt[:], AF.Prelu,
                                 bias=0.0, scale=1.0, alpha=prelu_a)
            o_t[s] = o

        def st_b(s):
            o = o_t.pop(s)
            wt = wps.tile([128, 1024], f32, tag="w", name=f"w{s}")
            nc.tensor.matmul(wt[:, 0:512], pb_t, o[:, 0:512],
                             start=True, stop=True)
            nc.tensor.matmul(wt[:, 512:1024], pb_t, o[:, 512:1024],
                             start=True, stop=True)
            rr = rrp.tile([128, 1024], h16, tag="rr", name=f"rr{s}")
            nc.vector.tensor_scalar(
                out=rr[:], in0=wt[:],
                scalar1=bf_t, scalar2=zero_t,
                op0=ALU.add, op1=ALU.max)
            rr_t[s] = rr

        def st_c(s):
            rr = rr_t.pop(s)
            t, j = s // 8, s % 8
            for q in (0, 1):
                nc.tensor.matmul(po_ps[32 * t:32 * t + 32, :],
                                 pc_t[2 * j + q],
                                 rr[:, q * 512:(q + 1) * 512],
                                 start=(j == 0 and q == 0),
                                 stop=(j == 7 and q == 1),
                                 tile_position=(0, 32 * t),
                                 skip_group_check=True)

        for s in range(NSUP + 2):
            if s < NSUP:
                st_a(s)
            if 1 <= s < NSUP + 1:
                st_b(s - 1)
            if s >= 2:
                st_c(s - 2)

        # single drain: copy + one 128KB DMA
        po_sb = posbp.tile([128, 512], h16, tag="po_sb", name="po_sb")
        nc.scalar.activation(po_sb[:], po_ps[:], AF.Identity,
                             bias=bo2_t, scale=1.0)
        nc.sync.dma_start(out=out_d[:], in_=po_sb[:])

    nc.compile()
    return nc


def _get_program(prelu_a):
    key = ("prog", float(prelu_a))
    if key not in _CACHE:
        _CACHE[key] = _build_program(prelu_a)
    return _CACHE[key]


def make_in_maps(x, mask, W_fs, b_fs, W_in, b_in, adj, W_gc, b_gc, W_lo, b_lo,
                 prelu_a, W_ro, b_ro, W_o1, b_o1, W_o2, b_o2):
    x = np.asarray(x, np.float32)
    mask_f = np.asarray(mask, np.float32)
    adj = np.asarray(adj, np.float32)

    folded = _fold_weights(np.asarray(W_fs), np.asarray(b_fs),
                           np.asarray(W_in), np.asarray(b_in),
                           np.asarray(W_gc), np.asarray(b_gc),
                           np.asarray(W_lo), np.asarray(b_lo),
                           float(prelu_a),
                           np.asarray(W_ro), np.asarray(b_ro),
                           np.asarray(W_o1), np.asarray(b_o1),
                           np.asarray(W_o2), np.asarray(b_o2), adj)

    # adj grouped mt-major: adjs[p, mt*1024+nt*128+j] = adj[nt*128+p, mt*128+j]
    adjs = np.ascontiguousarray(
        adj.astype(np.float16).reshape(8, 128, 8, 128).transpose(1, 2, 0, 3)
    ).reshape(128, 8192)
    shared = dict(adjs=adjs, cgrep=folded["cgrep"],
                  consts_h=folded["consts_h"], consts_f=folded["consts_f"],
                  ones=np.ones(BLK, np.float16))
    u_all = (x[:, 0] - float(np.asarray(b_fs)[0])) * mask_f[:, 0]  # [B, N, T]
    in_maps = []
    for b in range(B):
        m = dict(shared)
        uh = u_all[b].astype(np.float16)          # [N, T]
        mh = mask_f[b, 0].astype(np.float16)
        um = np.empty((128, 1024), np.float16)
        um[:, 0:512] = uh.reshape(8, 128, T).transpose(1, 0, 2).reshape(
            128, 512)
        um[:, 512:1024] = mh.reshape(8, 128, T).transpose(1, 0, 2).reshape(
            128, 512)
        m["um"] = um
        m["up"] = np.ascontiguousarray(uh.reshape(NT))
        m["mp"] = np.ascontiguousarray(mh.reshape(NT))
        in_maps.append(m)
    return in_maps, folded["prelu_a"]


def kernel(x, mask, W_fs, b_fs, W_in, b_in, adj, W_gc, b_gc, W_lo, b_lo,
           prelu_a, W_ro, b_ro, W_o1, b_o1, W_o2, b_o2):
    in_maps, a = make_in_maps(x, mask, W_fs, b_fs, W_in, b_in, adj, W_gc,
                              b_gc, W_lo, b_lo, prelu_a, W_ro, b_ro, W_o1,
                              b_o1, W_o2, b_o2)
    nc = _get_program(a)

    from concourse.bass_utils import run_bass_kernel_spmd
    res = run_bass_kernel_spmd(nc, in_maps, list(range(B)))

    out = np.empty((B, C, N, T), np.float32)
    for b in range(B):
        # device row 4s+2q+lane = superstep s, pair g=2s+q, lane in {0,1}
        dev = np.asarray(res.results[b]["out"]).reshape(NSUP, 2, 2, CHUNK)
        flat = np.empty(NT, np.float32)
        fl = flat.reshape(2, NPAIR, CHUNK)     # [lane, chunk, 512]
        for q in (0, 1):
            fl[0, q::2] = dev[:, q, 0]
            fl[1, q::2] = dev[:, q, 1]
        out[b, 0] = flat.reshape(N, T)
    return out  # fp16 device output upcast to f32 on assignment


# revision 32
# speedup vs baseline: 1.1537x; 1.1537x over previous
"""Bass/Trainium2 kernel for nn_BiGRIL (gnn_message_passing).

Algebraic structure (h == 0, C == 1):
  u    = (x - bfs)*m                    (host-computed input prep)
  z    = W0*x1 + W1*m + b_in            (rank-2 in channels)
  zg   = A^T z  ->  W0*ug + W1*mg + b_in'*cg   with ug = A^T u, mg = A^T m
  v    = PA6^T [u,m,ug,mg,cg,1]         (K=6 matmul; bias via ones-row)
  o    = PReLU(v)                       (ACT Prelu, psum->fp16)
  w    = PB^T o                         (pass B; PB = wro1 (x) W_o1 rank-1)
  rr   = relu(w + bias_f)               (ACT Relu-bias / DVE TS, psum->fp16)
  out  = W_o2 . rr + b_o2               (pass C; +b_o2 via final copy bias)

Superstep schedule: 2 chunk-pairs per superstep; pass A fills a fused
[128,1024] 2-bank psum tile consumed by ONE wide Prelu; pass B fills a
single rotating [128,1024] psum tile consumed by ONE wide relu; pass C
packs its 4 useful output rows per superstep into ONE accumulated psum
bank (rows 4s..4s+3 via 16 stationary column-variants + tile_position),
so the whole output drains with a single copy + a single 128KB DMA.

Inputs are host-preshuffled: u/m/cg ma-rows arrive position-major straight
from HBM; only device-computed ug/mg need on-chip gathers (gpsimd swdge).
adj is grouped mt-major so G-tiles start as stripes land.  Output is
written chunk-major and unscrambled on host.

Sharding: data-parallel over batch (B=8 -> 8 cores), no collectives.
"""

import numpy as np
import sys

sys.path.insert(0, "/opt/trn_rl_repo")

B, C, N, T = 8, 1, 1024, 64
H = 64
NT = N * T          # 65536 per-core output elements
CHUNK = 512
NPAIR = 64          # pairs (g, g+64); chunk c=g is lane0, c=g+64 lane1
NSUP = 32           # supersteps of 2 pairs
HALF = NT // 2      # 32768: lane1 position offset
BLK = 4096          # ma tile columns (64 nodes x 64 steps)
NBLK = 8            # blocks per lane

_CACHE = {}


def _fold_weights(W_fs, b_fs, W_in, b_in, W_gc, b_gc, W_lo, b_lo, prelu_a,
                  W_ro, b_ro, W_o1, b_o1, W_o2, b_o2, adj):
    """Host-side weight folding in float64 for accuracy."""
    f8 = np.float64
    W_in, b_in = W_in.astype(f8), b_in.astype(f8)
    W_gc, b_gc = W_gc.astype(f8), b_gc.astype(f8)
    W_lo, b_lo = W_lo.astype(f8), b_lo.astype(f8)
    W_ro, b_ro = W_ro.astype(f8), b_ro.astype(f8)
    W_o1, b_o1 = W_o1.astype(f8), b_o1.astype(f8)
    W_o2, b_o2 = W_o2.astype(f8), b_o2.astype(f8)

    W0 = W_in[:, 0]           # x1 channel  [64]
    W1 = W_in[:, 1]           # mask channel [64]
    Wlo1 = W_lo[:, :H]
    M1 = Wlo1 @ W_gc[:, :H]
    M2 = Wlo1 @ W_gc[:, H:]
    b_fold = Wlo1 @ b_gc + b_lo

    b_in_p = b_in + W0 * float(b_fs[0])
    PA = np.stack([
        M1 @ W0,
        M1 @ W1,
        M2 @ W0,
        M2 @ W1,
        M2 @ b_in_p,
    ])                                     # [5, 64]  lhsT for pass A
    bias_v = M1 @ b_in_p + b_fold          # [64] -> ones-row of pass A

    w_ro1 = W_ro[0, :H]                    # [64]
    PB = np.outer(w_ro1, W_o1[:, 0])       # [64(h), 64(f)] lhsT for pass B
    bias_f = W_o1[:, 0] * b_ro[0] + b_o1   # [64]

    cg = adj.astype(f8).sum(axis=0)        # [N] column sums of adj
    cgrep = np.repeat(cg, T)               # [(n,t)] layout n*T + t

    a = float(prelu_a)
    assert 0.0 < a < 1.0

    # pass A stationary [11, 128]: row 0 = shared ones stream carrying
    # bias_v for BOTH lanes; rows 1-5 = lane0 streams (u,m,ug,mg,cg) ->
    # out cols 0:64; rows 6-10 = lane1 streams -> out cols 64:128.
    # K=11 means the moving tile has NO dead rows -> no zero-memsets.
    paA = np.zeros((128, 128))
    paA[0, 0:64] = bias_v
    paA[0, 64:128] = bias_v
    paA[1:6, 0:64] = PA
    paA[6:11, 64:128] = PA
    # pass B stationary: blockdiag(PB, PB) over packed prelu [128, 512]
    pbB = np.zeros((128, 128))
    pbB[0:64, 0:64] = PB
    pbB[64:128, 64:128] = PB
    # pass C stationaries: 16 variants [128, 32]; variant k = 2j+q places
    # lane0 at col 4j+2q, lane1 at col 4j+2q+1 -> accumulated po-bank row
    # 32t + 4j + 2q + lane for superstep s = 8t+j, pair q
    pc = np.zeros((128, 16 * 32))
    for k in range(16):
        j, q = k // 2, k % 2
        pc[0:64, 32 * k + 4 * j + 2 * q] = W_o2[0]
        pc[64:128, 32 * k + 4 * j + 2 * q + 1] = W_o2[0]

    h16 = np.float16
    fp = np.float32
    ch = np.zeros((128, 768))
    ch[:, 0:128] = paA
    ch[:, 128:256] = pbB
    ch[:, 256:768] = pc
    cf = np.zeros((128, 4))
    cf[0:64, 0] = bias_f           # rr relu bias
    cf[64:128, 0] = bias_f
    cf[:, 1] = float(b_o2[0])      # final copy bias (all rows are outputs)
    cf[:, 2] = a                   # prelu slope (unused; ACT alpha)
    # col 3 stays zero: zero-scalar operand for DVE tensor_scalar ops

    return dict(
        consts_h=ch.astype(h16),
        consts_f=cf.astype(fp),
        cgrep=cgrep.astype(h16),
        prelu_a=a,
    )


def _build_program(prelu_a):
    import concourse.bass as bass
    import concourse.bacc as bacc
    import concourse.mybir as mybir
    import concourse.tile as tile

    dt = mybir.dt
    f32 = dt.float32
    h16 = dt.float16
    AF = mybir.ActivationFunctionType
    ALU = mybir.AluOpType

    nc = bacc.Bacc("TRN2", target_bir_lowering=False, debug=False,
                   num_devices=B)

    um_d = nc.dram_tensor("um", [128, 1024], h16, kind="ExternalInput")
    ones_d = nc.dram_tensor("ones", [BLK], h16, kind="ExternalInput")
    up_d = nc.dram_tensor("up", [NT], h16, kind="ExternalInput")
    mp_d = nc.dram_tensor("mp", [NT], h16, kind="ExternalInput")
    cg_d = nc.dram_tensor("cgrep", [NT], h16, kind="ExternalInput")
    adjs = nc.dram_tensor("adjs", [128, 8192], h16, kind="ExternalInput")
    ch_d = nc.dram_tensor("consts_h", [128, 768], h16, kind="ExternalInput")
    cf_d = nc.dram_tensor("consts_f", [128, 4], f32, kind="ExternalInput")
    # row-major [128, 512] final tile; row 4s+2q+lane = superstep s pair q
    out_d = nc.dram_tensor("out", [NT], h16, kind="ExternalOutput")

    MOVA_BUFS = 4
    from contextlib import ExitStack
    with tile.TileContext(nc) as tc, ExitStack() as ctx:
        const = ctx.enter_context(tc.tile_pool(name="const", bufs=1))
        adjp = ctx.enter_context(tc.tile_pool(name="adjp", bufs=1))
        ump = ctx.enter_context(tc.tile_pool(name="ump", bufs=1))
        gxp = ctx.enter_context(tc.tile_pool(name="gxp", bufs=1))
        movap = ctx.enter_context(tc.tile_pool(name="movap", bufs=1))
        op_p = ctx.enter_context(tc.tile_pool(name="op", bufs=3))
        rrp = ctx.enter_context(tc.tile_pool(name="rrp", bufs=2))
        posbp = ctx.enter_context(tc.tile_pool(name="posbp", bufs=1))
        # PSUM budget (8 banks): vps 2x2 + wps 3x1 + pop 1
        vps = ctx.enter_context(tc.tile_pool(name="vps", bufs=2, space="PSUM"))
        wps = ctx.enter_context(tc.tile_pool(name="wps", bufs=3, space="PSUM"))
        pop = ctx.enter_context(tc.tile_pool(name="pop", bufs=1, space="PSUM"))

        # ---- consolidated constant loads (2 DMAs, first on sync) ------
        ch_t = const.tile([128, 768], h16)
        cf_t = const.tile([128, 4], f32)
        nc.sync.dma_start(out=ch_t[:], in_=ch_d[:])
        nc.sync.dma_start(out=cf_t[:], in_=cf_d[:])
        pa_t = ch_t[0:11, 0:128]
        pb_t = ch_t[:, 128:256]
        pc_t = [ch_t[:, 256 + 32 * k:256 + 32 * (k + 1)] for k in range(16)]
        bf_t = cf_t[:, 0:1]
        bo2_t = cf_t[:, 1:2]
        zero_t = cf_t[:, 3:4]

        # ---- G moving operand: [u|m] node-major, one DMA ---------------
        um_t = ump.tile([128, 1024], h16, tag="um", name="um")
        nc.sync.dma_start(out=um_t[:], in_=um_d[:])
        gmh = um_t[:, :].rearrange("p (h q) -> p h q", h=2)

        # ---- adj: 8 mt-stripes; 0,4 now, rest staggered in st_a --------
        adjt = adjp.tile([128, 8192], h16, tag="adjt", name="adjt")

        def load_stripe(mt, eng):
            c0 = mt * 1024
            eng.dma_start(out=adjt[:, c0:c0 + 1024],
                          in_=adjs[:, c0:c0 + 1024])

        load_stripe(0, nc.sync)
        load_stripe(4, nc.scalar)

        # ---- HAM warmup into the pop bank (garbage harmless: each 32-row
        # group is zeroed by its first C matmul's start=True) -------------
        po_ps = pop.tile([128, 512], f32, tag="po", name="po")
        for wi in range(28):
            nc.tensor.matmul(po_ps[:, 0:128], pb_t, pb_t,
                             start=True, stop=True, skip_group_check=True)

        # ---- ma tiles: 4 rotating [11, 4096] fp16 ----------------------
        # row 0 = shared ones (DMA-loaded once); rows 1-5 lane0 streams
        # (u, m, ug, mg, cg); rows 6-10 lane1.  No dead rows, no memsets.
        ma4 = [movap.tile([11, BLK], h16, tag=f"mova{i}", name=f"mova{i}")
               for i in range(MOVA_BUFS)]
        for i in range(MOVA_BUFS):
            nc.sync.dma_start(out=ma4[i][0:1, :], in_=ones_d[:])

        gx = [None] * 8
        ma_t = [None] * NBLK

        def emit_g(mt):
            # G psum borrows a vps tile (cols 0:128 used)
            psg = vps.tile([128, 1024], f32, tag="v", name=f"psg{mt}")
            for nt in range(8):
                c0 = mt * 1024 + nt * 128
                nc.tensor.matmul(
                    psg[:, 0:128],
                    adjt[:, c0:c0 + 128],
                    gmh[:, :, nt * 64:(nt + 1) * 64],
                    start=(nt == 0), stop=(nt == 7))
            g = gxp.tile([128, 128], h16, tag=f"gxm{mt}", name=f"gxm{mt}")
            nc.scalar.activation(g[:], psg[:, 0:128], AF.Copy,
                                 bias=0.0, scale=1.0)
            gx[mt] = g

        def emit_ma(blk):
            mt0, mt1 = blk // 2, 4 + blk // 2
            p0 = (blk % 2) * 64
            ma = ma4[blk % MOVA_BUFS]
            o0, o1 = blk * BLK, HALF + blk * BLK
            # position-major HBM rows (u, m, cg per lane)
            nc.sync.dma_start(out=ma[1:2, :], in_=up_d[o0:o0 + BLK])
            nc.scalar.dma_start(out=ma[2:3, :], in_=mp_d[o0:o0 + BLK])
            nc.sync.dma_start(out=ma[5:6, :], in_=cg_d[o0:o0 + BLK])
            nc.sync.dma_start(out=ma[6:7, :], in_=up_d[o1:o1 + BLK])
            nc.scalar.dma_start(out=ma[7:8, :], in_=mp_d[o1:o1 + BLK])
            nc.sync.dma_start(out=ma[10:11, :], in_=cg_d[o1:o1 + BLK])
            # device-computed ug/mg gathers (split scalar hw-DGE / gpsimd)
            nc.scalar.dma_start(out=ma[3:4, :], in_=gx[mt0][p0:p0 + 64, 0:64])
            nc.scalar.dma_start(out=ma[4:5, :],
                                in_=gx[mt0][p0:p0 + 64, 64:128])
            nc.gpsimd.dma_start(out=ma[8:9, :],
                                in_=gx[mt1][p0:p0 + 64, 0:64])
            nc.gpsimd.dma_start(out=ma[9:10, :],
                                in_=gx[mt1][p0:p0 + 64, 64:128])
            ma_t[blk] = ma

        o_t = {}
        rr_t = {}

        def st_a(s):
            blk = s // 4
            if s == 0:
                emit_g(0)
                emit_g(4)
                emit_ma(0)
                emit_ma(1)
                load_stripe(1, nc.scalar)
                load_stripe(5, nc.sync)
            elif s == 1:
                load_stripe(2, nc.scalar)
                load_stripe(6, nc.sync)
            elif s == 2:
                load_stripe(3, nc.scalar)
                load_stripe(7, nc.sync)
            if s % 4 == 2 and blk % 2 == 0 and blk // 2 + 1 < 4:
                emit_g(blk // 2 + 1)
                emit_g(4 + blk // 2 + 1)
            if s % 4 == 3 and blk + 2 < NBLK:
                emit_ma(blk + 2)
            c0 = (s % 4) * 1024
            vt = vps.tile([128, 1024], f32, tag="v", name=f"v{s}")
            nc.tensor.matmul(vt[:, 0:512], pa_t,
                             ma_t[blk][:, c0:c0 + 512],
                             start=True, stop=True)
            nc.tensor.matmul(vt[:, 512:1024], pa_t,
                             ma_t[blk][:, c0 + 512:c0 + 1024],
                             start=True, stop=True)
            o = op_p.tile([128, 1024], h16, tag="o", name=f"o{s}")
            nc.scalar.activation(o[:], vt[:], AF.Prelu,
                                 bias=0.0, scale=1.0, alpha=prelu_a)
            o_t[s] = o

        def st_b(s):
            o = o_t.pop(s)
            rr = rrp.tile([128, 1024], h16, tag="rr", name=f"rr{s}")
            for q in (0, 1):
                wq = wps.tile([128, 512], f32, tag="w", name=f"w{s}_{q}")
                nc.tensor.matmul(wq[:], pb_t, o[:, q * 512:(q + 1) * 512],
                                 start=True, stop=True)
                g = 2 * s + q
                if g % 6 == 0:
                    nc.scalar.activation(rr[:, q * 512:(q + 1) * 512], wq[:],
                                         AF.Relu, bias=bf_t, scale=1.0)
                else:
                    nc.vector.tensor_scalar(
                        out=rr[:, q * 512:(q + 1) * 512], in0=wq[:],
                        scalar1=bf_t, scalar2=zero_t,
                        op0=ALU.add, op1=ALU.max)
            rr_t[s] = rr

        def st_c(s):
            rr = rr_t.pop(s)
            t, j = s // 8, s % 8
            for q in (0, 1):
                nc.tensor.matmul(po_ps[32 * t:32 * t + 32, :],
                                 pc_t[2 * j + q],
                                 rr[:, q * 512:(q + 1) * 512],
                                 start=(j == 0 and q == 0),
                                 stop=(j == 7 and q == 1),
                                 tile_position=(0, 32 * t),
                                 skip_group_check=True)

        for s in range(NSUP + 2):
            if s < NSUP:
                st_a(s)
            if 1 <= s < NSUP + 1:
                st_b(s - 1)
            if s >= 2:
                st_c(s - 2)

        # single drain: copy + one 128KB DMA
        po_sb = posbp.tile([128, 512], h16, tag="po_sb", name="po_sb")
        nc.scalar.activation(po_sb[:], po_ps[:], AF.Identity,
                             bias=bo2_t, scale=1.0)
        nc.sync.dma_start(out=out_d[:], in_=po_sb[:])

    nc.compile()
    return nc


def _get_program(prelu_a):
    key = ("prog", float(prelu_a))
    if key not in _CACHE:
        _CACHE[key] = _build_program(prelu_a)
    return _CACHE[key]


def make_in_maps(x, mask, W_fs, b_fs, W_in, b_in, adj, W_gc, b_gc, W_lo, b_lo,
                 prelu_a, W_ro, b_ro, W_o1, b_o1, W_o2, b_o2):
    x = np.asarray(x, np.float32)
    mask_f = np.asarray(mask, np.float32)
    adj = np.asarray(adj, np.float32)

    folded = _fold_weights(np.asarray(W_fs), np.asarray(b_fs),
                           np.asarray(W_in), np.asarray(b_in),
                           np.asarray(W_gc), np.asarray(b_gc),
                           np.asarray(W_lo), np.asarray(b_lo),
                           float(prelu_a),
                           np.asarray(W_ro), np.asarray(b_ro),
                           np.asarray(W_o1), np.asarray(b_o1),
                           np.asarray(W_o2), np.asarray(b_o2), adj)

    # adj grouped mt-major: adjs[p, mt*1024+nt*128+j] = adj[nt*128+p, mt*128+j]
    adjs = np.ascontiguousarray(
        adj.astype(np.float16).reshape(8, 128, 8, 128).transpose(1, 2, 0, 3)
    ).reshape(128, 8192)
    shared = dict(adjs=adjs, cgrep=folded["cgrep"],
                  consts_h=folded["consts_h"], consts_f=folded["consts_f"],
                  ones=np.ones(BLK, np.float16))
    u_all = (x[:, 0] - float(np.asarray(b_fs)[0])) * mask_f[:, 0]  # [B, N, T]
    in_maps = []
    for b in range(B):
        m = dict(shared)
        uh = u_all[b].astype(np.float16)          # [N, T]
        mh = mask_f[b, 0].astype(np.float16)
        um = np.empty((128, 1024), np.float16)
        um[:, 0:512] = uh.reshape(8, 128, T).transpose(1, 0, 2).reshape(
            128, 512)
        um[:, 512:1024] = mh.reshape(8, 128, T).transpose(1, 0, 2).reshape(
            128, 512)
        m["um"] = um
        m["up"] = np.ascontiguousarray(uh.reshape(NT))
        m["mp"] = np.ascontiguousarray(mh.reshape(NT))
        in_maps.append(m)
    return in_maps, folded["prelu_a"]


def kernel(x, mask, W_fs, b_fs, W_in, b_in, adj, W_gc, b_gc, W_lo, b_lo,
           prelu_a, W_ro, b_ro, W_o1, b_o1, W_o2, b_o2):
    in_maps, a = make_in_maps(x, mask, W_fs, b_fs, W_in, b_in, adj, W_gc,
                              b_gc, W_lo, b_lo, prelu_a, W_ro, b_ro, W_o1,
                              b_o1, W_o2, b_o2)
    nc = _get_program(a)

    from concourse.bass_utils import run_bass_kernel_spmd
    res = run_bass_kernel_spmd(nc, in_maps, list(range(B)))

    out = np.empty((B, C, N, T), np.float32)
    for b in range(B):
        # device row 4s+2q+lane = superstep s, pair g=2s+q, lane in {0,1}
        dev = np.asarray(res.results[b]["out"]).reshape(NSUP, 2, 2, CHUNK)
        flat = np.empty(NT, np.float32)
        fl = flat.reshape(2, NPAIR, CHUNK)     # [lane, chunk, 512]
        for q in (0, 1):
            fl[0, q::2] = dev[:, q, 0]
            fl[1, q::2] = dev[:, q, 1]
        out[b, 0] = flat.reshape(N, T)
    return out  # fp16 device output upcast to f32 on assignment
